# revision 1
# baseline (speedup 1.0000x reference)
"""Trainium2 Bass kernel for DecouplingSpecificSpecificLoss.

Reference computation: reshape [16384, 2048] -> [4096 chunks, 4 views, 2048],
L2-normalize rows, per-chunk 4x4 cosine-similarity matrix, clip to
[5e-4, 0.9995], loss = sum over chunks of mean(-log(1 - sim)).

Strategy (8 NeuronCores, data parallel over chunks):
  - Each core gets 2048 contiguous rows (512 chunks of 4 rows).
  - Layout: one chunk per SBUF partition -> segments of [128 partitions,
    4 views * w]. Each partition's bytes are contiguous-per-view in HBM,
    so the loads run near the HBM line rate. The stream is staircased
    (d-widths 512/768/768, then 1024-halves, then full 2048 tiles) so
    VectorE starts ~13us in and never starves.
  - Per segment the heavy math is 10 dot products per chunk:
      * 4 self-dots (squared norms): ScalarE activation(Square, accum_out)
      * 6 cross-dots on VectorE via fused scalar_tensor_tensor(accum_out)
        (f32 on purpose: the op has only a 1x uop, so bf16 buys nothing,
        and GpSimd offload stalls DVE via the shared SBUF port pair)
  - Tiny [128, 10] result per segment is DMA'd out on the Scalar HWDGE
    ring (the Sync ring carries the loads; a compute-gated store there
    would stall later load dispatch). The host finishes the
    normalize/clip/log/reduce on ~71k scalars. The diagonal of each 4x4
    sim matrix is always 1 -> clips to 0.9995, so it contributes a closed
    form constant and is never computed on device.
  - Measured: ~72us HW exec per core vs a ~47us pure-DMA roofline; the
    binding constraint is VectorE's 2-read-port limit (53us of fused dot
    work), which runs gap-free behind the DMA stream.
"""

import json
import sys

if "/opt/trn_rl_repo" not in sys.path:
    sys.path.insert(0, "/opt/trn_rl_repo")

import numpy as np

import concourse.bass as bass
import concourse.mybir as mybir
import concourse.tile as tile
from concourse.bass_utils import run_bass_kernel_spmd

N_CORES = 8
B, D = 16384, 2048
V = 4                                  # views (rows) per chunk
ROWS_PER_CORE = B // N_CORES           # 2048
CHUNKS_PER_CORE = ROWS_PER_CORE // V   # 512
P = 128                                # SBUF partitions
TILES = CHUNKS_PER_CORE // P           # 4
FREE = V * D                           # 8192 f32 per partition

CLAMP_MIN = 0.0005
CLAMP_MAX = 0.9995
NORM_EPS = 1e-12

# All six (view_i, view_j) cross pairs run as fused dot products on VectorE.
# (GpSimd offload was measured counterproductive: the f32 fused op occupies
# both DVE read ports every cycle, and GpSimd's SBUF door IS the shared
# second port, so concurrent GpSimd work stalls DVE 2-3x per instruction.)
ALL_PAIRS = [(0, 1), (0, 2), (0, 3), (1, 2), (1, 3), (2, 3)]

# Input stream segments: (row_start, d_lo, d_hi). The first row block is
# split along d (quarter, quarter, half) so compute starts as soon as the
# first ~1 MiB lands; later blocks stream as 2 MiB halves so VectorE never
# starves waiting for a whole 4 MiB tile.
SEGMENTS = [
    (0, 0, 512),
    (0, 512, 1280),
    (0, 1280, 2048),
    (512, 0, 1024),
    (512, 1024, 2048),
    (1024, 0, 2048),
    (1536, 0, 2048),
]


def build_bass():
    f32 = mybir.dt.float32
    nc = bass.Bass()
    x = nc.declare_dram_parameter("x", [ROWS_PER_CORE, D], f32, isOutput=False)
    out = nc.declare_dram_parameter(
        "out", [len(SEGMENTS), P, 10], f32, isOutput=True
    )

    with tile.TileContext(nc) as tc:
        with (
            tc.tile_pool(name="xpart", bufs=5) as xpart_pool,
            tc.tile_pool(name="sdve", bufs=2) as sdve_pool,
            tc.tile_pool(name="sact", bufs=2) as sact_pool,
            tc.tile_pool(name="acc", bufs=6) as acc_pool,
        ):
            for si, (r0, dlo, dhi) in enumerate(SEGMENTS):
                w = dhi - dlo
                xt = xpart_pool.tile([P, V * w], f32, tag="xp")
                src = x[r0 : r0 + 512, dlo:dhi].rearrange(
                    "(p r) d -> p r d", r=V
                )
                dst = xt[:].rearrange("p (r d) -> p r d", r=V)
                nc.sync.dma_start(dst, src)

                views = [xt[:, w * v : w * (v + 1)] for v in range(V)]
                acc = acc_pool.tile([P, 10], f32, tag="acc")
                dots = acc[:, 0:6]
                norms2 = acc[:, 6:10]

                # squared norms on ScalarE (fused square + accumulate)
                for v in range(V):
                    s = sact_pool.tile([P, D], f32, tag="sact")
                    nc.scalar.activation(
                        s[:, :w],
                        views[v],
                        mybir.ActivationFunctionType.Square,
                        accum_out=norms2[:, v : v + 1],
                    )

                # cross-dots on VectorE: fused multiply + free-axis accumulate
                for k, (a, b) in enumerate(ALL_PAIRS):
                    s = sdve_pool.tile([P, D], f32, tag="sdve")
                    nc.vector.scalar_tensor_tensor(
                        out=s[:, :w],
                        in0=views[a],
                        scalar=0.0,
                        in1=views[b],
                        op0=mybir.AluOpType.bypass,
                        op1=mybir.AluOpType.mult,
                        accum_out=dots[:, k : k + 1],
                    )

                # result DMA goes out on the Scalar engine's HWDGE ring:
                # it waits on compute, and on the Sync ring that FIFO wait
                # would stall dispatch of the next input load.
                nc.scalar.dma_start(out[si, :, :], acc[:])

    return nc


def _split_multiwait_bir(bir_json: bytes) -> bytes:
    """Legalize BIR for this walrus build: it rejects instructions carrying
    more than one semaphore wait ("Too many sync wait commands"). Tile emits
    multi-wait instructions (the tail Drain waits on every live sem; compute
    ops can wait on several producers). Hoist all but one wait onto fresh
    standalone EventSemaphore instructions inserted just before the original
    on the same engine — the engine sequencer executes them in order, so the
    semantics are unchanged.
    """
    mod = json.loads(bir_json)
    n_new = 0
    for fn in mod["functions"]:
        for bb in fn["blocks"]:
            out_insts = []
            for inst in bb["instructions"]:
                si = inst.get("sync_info") or {}
                waits = si.get("on_wait") or []
                cap = 2 if inst.get("opcode") == "EventSemaphore" else 1
                if len(waits) > cap:
                    keep = waits[: cap - 1] if cap > 1 else []
                    hoist = waits[len(keep) : -1]
                    last = [waits[-1]]
                    for w in hoist:
                        n_new += 1
                        out_insts.append(
                            {
                                "debug": inst.get("debug", 0),
                                "engine": inst["engine"],
                                "ins": [],
                                "name": f"{inst['name']}-hw{n_new}",
                                "opcode": "EventSemaphore",
                                "outs": [],
                                "sync_info": {"on_update": [], "on_wait": [w]},
                            }
                        )
                    si["on_wait"] = keep + last
                out_insts.append(inst)
            bb["instructions"] = out_insts
    return json.dumps(mod).encode()


_NC_CACHE = None


def _get_nc():
    global _NC_CACHE
    if _NC_CACHE is None:
        nc = build_bass()
        fixed = _split_multiwait_bir(nc.to_json_bytes())
        nc.to_json_bytes = lambda: fixed
        _NC_CACHE = nc
    return _NC_CACHE


def run(specific_features, trace=False, **trace_kw):
    """Run the device kernel; returns (per-core raw outputs, BassKernelResults)."""
    xs = np.asarray(specific_features, dtype=np.float32)
    assert xs.shape == (B, D), xs.shape
    shards = [
        np.ascontiguousarray(xs[c * ROWS_PER_CORE : (c + 1) * ROWS_PER_CORE])
        for c in range(N_CORES)
    ]
    in_maps = [{"x": s} for s in shards]
    nc = _get_nc()
    res = run_bass_kernel_spmd(
        nc, in_maps, list(range(N_CORES)), trace=trace, **trace_kw
    )
    outs = [r["out"] for r in res.results]
    return outs, res


def postprocess(outs):
    """Finish the loss from per-core [n_seg, P, 10] raw dot/norm tensors."""
    # f32-faithful diagonal term: sim_ii == 1 always clips to CLAMP_MAX.
    diag_term = float(-np.log(np.float32(1.0) - np.float32(CLAMP_MAX)))
    ii = [p[0] for p in ALL_PAIRS]
    jj = [p[1] for p in ALL_PAIRS]
    total = 0.0
    for arr in outs:
        a = np.asarray(arr, dtype=np.float64)  # [n_seg, P, 10]
        seg_n2 = a[..., 6:10]
        seg_dots = a[..., 0:6]
        dots = np.zeros((CHUNKS_PER_CORE, 6))
        n2 = np.zeros((CHUNKS_PER_CORE, V))
        for si, (r0, _dlo, _dhi) in enumerate(SEGMENTS):
            c0 = r0 // V
            dots[c0 : c0 + P] += seg_dots[si]
            n2[c0 : c0 + P] += seg_n2[si]
        norms = np.maximum(np.sqrt(n2), NORM_EPS)
        sim = dots / (norms[:, ii] * norms[:, jj])
        sim = np.clip(sim, CLAMP_MIN, CLAMP_MAX)
        # each unordered pair appears twice in the symmetric 4x4 matrix
        total += 2.0 * np.sum(-np.log1p(-sim))
    total += (B // V) * V * diag_term
    return np.float32(total / (V * V))


def kernel(specific_features):
    outs, _ = run(specific_features, trace=False)
    return postprocess(outs)


if __name__ == "__main__":
    x = np.random.default_rng(0).standard_normal((B, D)).astype(np.float32)
    print(kernel(x))

